# revision 1
# baseline (speedup 1.0000x reference)
"""Trainium2 Bass kernel for nn_DConv (diffusion graph conv, K=2, 2 supports).

Contract: kernel(**inputs) takes FULL unsharded inputs (inputs [B,N,D] f32,
adj_vals [E] f32, rows/cols [E] int, weights [D*M,OUT] f32, biases [1,OUT]
f32) and returns the FULL output [B, N, OUT] f32.

Strategy (data-parallel over batch, per the sharding hint):
 - Each of the 8 cores handles B/8 batches: x layout [N, D*Bl] (col = d*Bl+b).
 - Host builds the two normalized supports (vals1,rows->cols / vals2,cols->rows),
   sorts each edge list by destination into 128-node blocks, pads each block's
   edge segment to a multiple of 128 "slots".
 - Device, per spmm: dma_gather (bf16, 512B rows) fetches x[src] per slot;
   a per-chunk [128,128] selection matrix Sel[e, dst_local] = v_e (built on
   the vector engine as (iota==dst)*v) reduces each chunk into PSUM via
   TensorE: y_block += Sel^T @ Z. Eviction is a plain PSUM->bf16 copy; the
   Chebyshev recurrence (x2 = 2*S*x1 - x0) is folded into the projection
   weights on the host, so the 4 spmms produce raw S-products only:
     A1 = S1 X0, R2 = S1 A1, B1 = S2 A1, R4 = S2 B1
   out = X0(W0-W2) + A1(W1-W4) + R2(2 W2) + B1 W3 + R4(2 W4) + bias.
 - Projection: DMA-transpose loads X_m^T tiles, TensorE contracts against a
   host-built block-diagonal W~ [1280, OUT*Bl].
"""
import os
import sys
import numpy as np
import ml_dtypes

for _p in ('/opt/trn_rl_repo', '/root/.axon_site/_ro/trn_rl_repo'):
    if os.path.isdir(_p) and _p not in sys.path:
        sys.path.append(_p)

import concourse.bass as bass
import concourse.mybir as mybir
import concourse.tile as tile
from concourse import bacc
from concourse.bass_utils import run_bass_kernel_spmd

BF16 = ml_dtypes.bfloat16
P = 128
NCORES = 8


# ---------------------------------------------------------------- host prep

def _build_support(vals, src, dst, n_nodes):
    """Sort edges by dst, pad each 128-node block segment to a multiple of
    128 slots. Returns slot arrays + chunk metadata."""
    nb = n_nodes // P
    order = np.argsort(dst, kind='stable')
    s_src = src[order]
    s_dst = dst[order]
    s_v = vals[order]
    blk = (s_dst // P).astype(np.int64)
    cnt = np.bincount(blk, minlength=nb)

    src_parts, dstl_parts, v_parts = [], [], []
    chunk_block = []
    pos = 0
    for b in range(nb):
        c = int(cnt[b])
        nchunk = max(1, -(-c // P))
        pad = nchunk * P - c
        src_parts.append(s_src[pos:pos + c])
        dstl_parts.append(s_dst[pos:pos + c] - b * P)
        v_parts.append(s_v[pos:pos + c])
        if pad:
            src_parts.append(np.zeros(pad, s_src.dtype))
            dstl_parts.append(np.zeros(pad, s_dst.dtype))
            v_parts.append(np.zeros(pad, np.float32))
        chunk_block += [b] * nchunk
        pos += c

    slot_src = np.concatenate(src_parts).astype(np.int16)
    slot_dstl = np.concatenate(dstl_parts).astype(np.float32)
    slot_v = np.concatenate(v_parts).astype(np.float32)
    n_chunks = len(chunk_block)

    # slot-major [128, n_chunks]: arr[p, c] = val[c*128 + p]
    dst_t = np.ascontiguousarray(slot_dstl.reshape(n_chunks, P).T)
    v_t = np.ascontiguousarray(slot_v.reshape(n_chunks, P).T)

    # wrapped idx layout [128, n_slots/16]: tile[p, j] = idx[j*16 + p%16]
    idx = slot_src.reshape(-1, 16).T  # [16, n_slots/16]
    idx_w = np.ascontiguousarray(np.tile(idx, (8, 1)))

    # chunk -> (block, first, last)
    chunk_block = np.asarray(chunk_block)
    first = np.ones(n_chunks, bool)
    first[1:] = chunk_block[1:] != chunk_block[:-1]
    last = np.ones(n_chunks, bool)
    last[:-1] = chunk_block[:-1] != chunk_block[1:]
    return dict(idx_w=idx_w, dst_t=dst_t, v_t=v_t,
                chunk_block=chunk_block, first=first, last=last,
                n_chunks=n_chunks)


def preprocess(adj_vals, rows, cols, n_nodes):
    drow = np.zeros(n_nodes, np.float32)
    np.add.at(drow, rows, adj_vals)
    dcol = np.zeros(n_nodes, np.float32)
    np.add.at(dcol, cols, adj_vals)
    inv_drow = np.where(drow > 0, 1.0 / drow, 0.0).astype(np.float32)
    inv_dcol = np.where(dcol > 0, 1.0 / dcol, 0.0).astype(np.float32)
    vals1 = (adj_vals * inv_drow[rows]).astype(np.float32)
    vals2 = (adj_vals * inv_dcol[cols]).astype(np.float32)
    s1 = _build_support(vals1, rows, cols, n_nodes)
    s2 = _build_support(vals2, cols, rows, n_nodes)
    return s1, s2


def build_wtilde(weights, d_in, n_mat, out_dim, bl):
    """W~ [5*d_in*bl, out_dim*bl] bf16 with recurrence folded in.
    Row r = m*(d_in*bl) + (d*bl + b); col = o*bl + b."""
    W = weights.reshape(d_in, n_mat, out_dim)
    C = [W[:, 0] - W[:, 2], W[:, 1] - W[:, 4], 2.0 * W[:, 2], W[:, 3], 2.0 * W[:, 4]]
    F = d_in * bl
    Wt = np.zeros((5 * F, out_dim * bl), np.float32)
    for m in range(5):
        for d in range(d_in):
            for b in range(bl):
                Wt[m * F + d * bl + b, b::bl] = C[m][d]
    return Wt.astype(BF16)


# ---------------------------------------------------------------- program

def build_program(n_nodes, feat, out_feat, sup_metas, call_chunks=64, selg=8):
    """Build the per-core Bass program. sup_metas = (s1, s2) chunk metadata
    (only n_chunks/chunk_block/first/last are used — the program layout
    depends on them)."""
    ob = 256  # out_dim * bl
    nt = n_nodes // P  # projection node tiles
    n_wchunks = 5 * feat // P

    nc = bacc.Bacc("TRN2", target_bir_lowering=False, debug=False,
                   num_devices=NCORES)
    dt = mybir.dt

    x0 = nc.dram_tensor("x0", [n_nodes, feat], dt.bfloat16, kind="ExternalInput")
    iota_in = nc.dram_tensor("iota", [P, P], dt.float32, kind="ExternalInput")
    wt_in = nc.dram_tensor("wt", [5 * feat, ob], dt.bfloat16, kind="ExternalInput")
    bias_in = nc.dram_tensor("bias", [P, ob], dt.float32, kind="ExternalInput")

    sup_t = []
    for i, s in enumerate(sup_metas):
        n_slots = s['n_chunks'] * P
        sup_t.append(dict(
            idx=nc.dram_tensor(f"idx{i}", [P, n_slots // 16], dt.int16,
                               kind="ExternalInput"),
            dst=nc.dram_tensor(f"dst{i}", [P, s['n_chunks']], dt.float32,
                               kind="ExternalInput"),
            v=nc.dram_tensor(f"v{i}", [P, s['n_chunks']], dt.float32,
                             kind="ExternalInput"),
        ))

    A1 = nc.dram_tensor("A1", [n_nodes, feat], dt.bfloat16, kind="Internal")
    R2 = nc.dram_tensor("R2", [n_nodes, feat], dt.bfloat16, kind="Internal")
    B1 = nc.dram_tensor("B1", [n_nodes, feat], dt.bfloat16, kind="Internal")
    R4 = nc.dram_tensor("R4", [n_nodes, feat], dt.bfloat16, kind="Internal")
    out = nc.dram_tensor("out", [n_nodes, ob], dt.float32, kind="ExternalOutput")

    with tile.TileContext(nc) as tc:
        with (
            tc.tile_pool(name="const", bufs=1) as cpool,
            tc.tile_pool(name="z", bufs=2) as zpool,
            tc.tile_pool(name="idx", bufs=2) as ipool,
            tc.tile_pool(name="dv", bufs=2) as dvpool,
            tc.tile_pool(name="sel", bufs=2) as selpool,
            tc.tile_pool(name="ev", bufs=4) as evpool,
            tc.tile_pool(name="lhs", bufs=2) as lpool,
            tc.tile_pool(name="po", bufs=2) as opool,
            tc.tile_pool(name="ps", bufs=4, space="PSUM") as pspool,
            tc.tile_pool(name="pso", bufs=2, space="PSUM") as psopool,
        ):
            iota_sb = cpool.tile([P, P], dt.float32)
            nc.sync.dma_start(iota_sb[:], iota_in[:, :])
            wt_sb = cpool.tile([P, n_wchunks, ob], dt.bfloat16)
            nc.sync.dma_start(
                wt_sb[:],
                wt_in[:, :].rearrange("(k p) o -> p k o", p=P))
            bias_sb = cpool.tile([P, ob], dt.float32)
            nc.sync.dma_start(bias_sb[:], bias_in[:, :])

            def emit_spmm(sup, st, xsrc, ydst):
                n_chunks = sup['n_chunks']
                cb = sup['chunk_block']
                first = sup['first']
                last = sup['last']
                ps = None
                for c0 in range(0, n_chunks, call_chunks):
                    ncall = min(call_chunks, n_chunks - c0)
                    nidx = ncall * P
                    idx_t = ipool.tile([P, call_chunks * 8], dt.int16, tag="idx")
                    nc.sync.dma_start(
                        idx_t[:, :ncall * 8],
                        st['idx'][:, c0 * 8:(c0 + ncall) * 8])
                    dst_t = dvpool.tile([P, call_chunks], dt.float32, tag="dst")
                    nc.sync.dma_start(dst_t[:, :ncall],
                                      st['dst'][:, c0:c0 + ncall])
                    v_t = dvpool.tile([P, call_chunks], dt.float32, tag="v")
                    nc.sync.dma_start(v_t[:, :ncall],
                                      st['v'][:, c0:c0 + ncall])
                    z_t = zpool.tile([P, call_chunks, feat], dt.bfloat16, tag="z")
                    nc.gpsimd.dma_gather(
                        z_t[:, :ncall, :], xsrc[:, :], idx_t[:, :ncall * 8],
                        nidx, nidx, feat, single_packet=False)
                    sel_t = selpool.tile([P, call_chunks, P], dt.bfloat16,
                                         tag="sel")
                    for g0 in range(0, ncall, selg):
                        ng = min(selg, ncall - g0)
                        sel_sl = sel_t[:, g0:g0 + ng, :]
                        nc.vector.tensor_tensor(
                            out=sel_sl,
                            in0=iota_sb[:][:, None, :].to_broadcast([P, ng, P]),
                            in1=dst_t[:, g0:g0 + ng, None].to_broadcast([P, ng, P]),
                            op=mybir.AluOpType.is_equal)
                        nc.vector.tensor_tensor(
                            out=sel_sl,
                            in0=sel_sl,
                            in1=v_t[:, g0:g0 + ng, None].to_broadcast([P, ng, P]),
                            op=mybir.AluOpType.mult)
                    for cl in range(ncall):
                        c = c0 + cl
                        if first[c]:
                            ps = pspool.tile([P, feat], dt.float32, tag="ps")
                        nc.tensor.matmul(
                            out=ps[:],
                            lhsT=sel_t[:, cl, :],
                            rhs=z_t[:, cl, :],
                            start=bool(first[c]),
                            stop=bool(last[c]),
                        )
                        if last[c]:
                            b = cb[c]
                            y_sb = evpool.tile([P, feat], dt.bfloat16, tag="y")
                            nc.vector.tensor_copy(out=y_sb[:], in_=ps[:])
                            nc.sync.dma_start(
                                ydst[b * P:(b + 1) * P, :], y_sb[:])

            emit_spmm(sup_metas[0], sup_t[0], x0, A1)
            emit_spmm(sup_metas[0], sup_t[0], A1, R2)
            emit_spmm(sup_metas[1], sup_t[1], A1, B1)
            emit_spmm(sup_metas[1], sup_t[1], B1, R4)

            # projection
            xs = [x0, A1, R2, B1, R4]
            for t in range(nt):
                rows = slice(t * P, (t + 1) * P)
                pso = psopool.tile([P, ob], dt.float32, tag="pso")
                for k in range(n_wchunks):
                    m, h = divmod(k, feat // P)
                    lhsT = lpool.tile([P, P], dt.bfloat16, tag="lhsT")
                    nc.sync.dma_start_transpose(
                        lhsT[:], xs[m][rows, h * P:(h + 1) * P])
                    nc.tensor.matmul(
                        out=pso[:],
                        lhsT=lhsT[:],
                        rhs=wt_sb[:, k, :],
                        start=(k == 0),
                        stop=(k == n_wchunks - 1),
                    )
                o_sb = opool.tile([P, ob], dt.float32, tag="osb")
                nc.vector.tensor_tensor(out=o_sb[:], in0=pso[:],
                                        in1=bias_sb[:],
                                        op=mybir.AluOpType.add)
                nc.sync.dma_start(out[rows, :], o_sb[:])

    nc.compile()
    return nc


# ---------------------------------------------------------------- entry

def _make_core_inputs(core, inputs_f32, s1, s2, wt, bias_rep, n_nodes, d_in):
    bl = inputs_f32.shape[0] // NCORES
    x0 = np.ascontiguousarray(
        inputs_f32[core * bl:(core + 1) * bl]
        .transpose(1, 2, 0).reshape(n_nodes, d_in * bl)).astype(BF16)
    iota = np.tile(np.arange(P, dtype=np.float32)[None, :], (P, 1))
    return dict(
        x0=x0, iota=iota, wt=wt, bias=bias_rep,
        idx0=s1['idx_w'], dst0=s1['dst_t'], v0=s1['v_t'],
        idx1=s2['idx_w'], dst1=s2['dst_t'], v1=s2['v_t'],
    )


def kernel(**inputs):
    inputs_f32 = np.asarray(inputs['inputs'], dtype=np.float32)
    adj_vals = np.asarray(inputs['adj_vals'], dtype=np.float32)
    rows = np.asarray(inputs['rows']).astype(np.int64)
    cols = np.asarray(inputs['cols']).astype(np.int64)
    weights = np.asarray(inputs['weights'], dtype=np.float32)
    biases = np.asarray(inputs['biases'], dtype=np.float32)

    b_total, n_nodes, d_in = inputs_f32.shape
    out_dim = weights.shape[1]
    n_mat = weights.shape[0] // d_in
    bl = b_total // NCORES
    assert n_mat == 5, "kernel is specialized for K=2 (M=5)"

    s1, s2 = preprocess(adj_vals, rows, cols, n_nodes)
    wt = build_wtilde(weights, d_in, n_mat, out_dim, bl)
    bias_rep = np.zeros((P, out_dim * bl), np.float32)
    for o in range(out_dim):
        bias_rep[:, o * bl:(o + 1) * bl] = biases[0, o]

    nc = build_program(n_nodes, d_in * bl, out_dim, (s1, s2))

    in_maps = [
        _make_core_inputs(c, inputs_f32, s1, s2, wt, bias_rep, n_nodes, d_in)
        for c in range(NCORES)
    ]
    res = run_bass_kernel_spmd(nc, in_maps, core_ids=list(range(NCORES)))

    out = np.zeros((b_total, n_nodes, out_dim), np.float32)
    for c in range(NCORES):
        oc = res.results[c]['out']  # [n_nodes, out*bl], col = o*bl + b
        out[c * bl:(c + 1) * bl] = (
            oc.reshape(n_nodes, out_dim, bl).transpose(2, 0, 1))
    return out



# revision 3
# speedup vs baseline: 3.1150x; 3.1150x over previous
"""Trainium2 Bass kernel for nn_DConv (diffusion graph conv, K=2, 2 supports).

Contract: kernel(**inputs) takes FULL unsharded inputs (inputs [B,N,D] f32,
adj_vals [E] f32, rows/cols [E] int, weights [D*M,OUT] f32, biases [1,OUT]
f32) and returns the FULL output [B, N, OUT] f32.

Strategy (v1: dst-node-sharded edges + AllGather, all batches per core):
 - The bottleneck in the batch-parallel design was GpSimd (Q7) SWDGE
   descriptor generation for dma_gather at ~7.6ns/index: each core gathered
   all E=524288 edges per spmm. Here each core owns 1/8 of the DESTINATION
   nodes (rows [2048c, 2048(c+1))) and processes only the ~E/8 edges that
   land there, but carries ALL 32 batches (row width 64*32 = 2048 bf16 =
   4KB). Same DMA bytes per core, 8x fewer gather descriptors.
 - x tables are full [16384, 2048] bf16 (64MB). Column order: col =
   g*256 + d*4 + b' where b = 4g + b' (8 batch-groups of 4) so the
   projection reuses the baseline 256-col block-diagonal W~ per group.
 - Per spmm: sort own edges by dst into 16 local 128-node blocks, pad each
   block segment to a UNIFORM (across cores) number of 128-slot chunks so
   one SPMD program fits all cores. dma_gather (4KB rows) fetches x[src]
   per slot; DVE builds Sel[e, dst_local] = (iota==dst)*v in bf16; TensorE
   scatters chunk into 4 PSUM sections [128,512] f32 per dst block.
 - Chebyshev recurrence folded into projection weights (host): raw products
   A1 = S1 X0, B1 = S2 A1, R2 = S1 A1, R4 = S2 B1;
   out = X0(W0-W2) + A1(W1-W4) + R2(2 W2) + B1 W3 + R4(2 W4) + bias.
 - Two AllGathers move A1 and B1 shards (8MB each) to full tables (the only
   cross-spmm tables that are gathered from). R2/R4 stay local. B1 is
   emitted before R2 so AllGather(B1) overlaps R2's gathers.
 - Projection: per node tile (16) x batch group (8): DMA-transpose loads
   x_m^T col-slices, TensorE contracts against block-diag W~ [1280, 256].
"""
import os
import sys
import numpy as np
import ml_dtypes

for _p in ('/opt/trn_rl_repo', '/root/.axon_site/_ro/trn_rl_repo'):
    if os.path.isdir(_p) and _p not in sys.path:
        sys.path.append(_p)

import concourse.bass as bass
import concourse.mybir as mybir
import concourse.tile as tile
from concourse import bacc
from concourse.bass_utils import run_bass_kernel_spmd

BF16 = ml_dtypes.bfloat16
P = 128
NCORES = 8
N = 16384
SHARD = N // NCORES          # 2048 nodes per core
NBLK = SHARD // P            # 16 local dst blocks per core
D = 64
BTOT = 32
BL = 4                       # batches per projection group
NGRP = BTOT // BL            # 8 groups
FEAT = D * BTOT              # 2048 columns (4KB bf16 rows)
OUT = 64
OB = OUT * BL                # 256 out cols per group
CALL = 16                    # chunks per dma_gather call (2048 indices)
SELG = 8                     # chunks per DVE sel-build op
NSEC = 4                     # PSUM sections per block (4 x [128,512] f32)
SEC = FEAT // NSEC           # 512


# ---------------------------------------------------------------- host prep

def _shard_support(vals, src, dst):
    """Partition edges by dst block range of 2048 nodes per core; sort each
    core's edges by dst; pad each local 128-node block segment to a number of
    chunks that is UNIFORM across cores (program structure must be SPMD).

    Returns (meta, percore) where meta has the uniform chunk structure and
    percore[c] has idx_w/dst_t/v_t slot arrays."""
    per_core_sorted = []
    cnt = np.zeros((NCORES, NBLK), np.int64)
    for c in range(NCORES):
        lo, hi = SHARD * c, SHARD * (c + 1)
        m = (dst >= lo) & (dst < hi)
        s_dst = dst[m] - lo
        s_src = src[m]
        s_v = vals[m]
        order = np.argsort(s_dst, kind='stable')
        s_dst = s_dst[order]
        s_src = s_src[order]
        s_v = s_v[order]
        blk = s_dst // P
        cnt[c] = np.bincount(blk, minlength=NBLK)
        per_core_sorted.append((s_src, s_dst, s_v))

    cpb = np.maximum(1, -(-cnt.max(axis=0) // P))  # chunks per block (uniform)
    ch = int(cpb.sum())
    pad = (-ch) % CALL
    cpb[NBLK - 1] += pad
    ch += pad

    chunk_block = np.repeat(np.arange(NBLK), cpb)
    first = np.ones(ch, bool)
    first[1:] = chunk_block[1:] != chunk_block[:-1]
    last = np.ones(ch, bool)
    last[:-1] = chunk_block[:-1] != chunk_block[1:]
    meta = dict(ch=ch, cpb=cpb, chunk_block=chunk_block, first=first, last=last)

    percore = []
    for c in range(NCORES):
        s_src, s_dst, s_v = per_core_sorted[c]
        n_slots = ch * P
        slot_src = np.zeros(n_slots, np.int16)
        slot_dstl = np.zeros(n_slots, np.float32)
        slot_v = np.zeros(n_slots, np.float32)
        pos = 0
        off = 0
        for b in range(NBLK):
            n = int(cnt[c, b])
            sl = slice(off, off + n)
            slot_src[sl] = s_src[pos:pos + n].astype(np.int16)
            slot_dstl[sl] = (s_dst[pos:pos + n] - b * P).astype(np.float32)
            slot_v[sl] = s_v[pos:pos + n]
            pos += n
            off += int(cpb[b]) * P
        # wrapped idx layout [128, n_slots/16]: col j = slots [16j, 16j+16)
        idx_w = np.ascontiguousarray(
            np.tile(slot_src.reshape(-1, 16).T, (8, 1)))
        dst_t = np.ascontiguousarray(
            slot_dstl.reshape(ch, P).T).astype(BF16)
        v_t = np.ascontiguousarray(slot_v.reshape(ch, P).T).astype(BF16)
        percore.append(dict(idx_w=idx_w, dst_t=dst_t, v_t=v_t))
    return meta, percore


def preprocess(adj_vals, rows, cols):
    drow = np.zeros(N, np.float32)
    np.add.at(drow, rows, adj_vals)
    dcol = np.zeros(N, np.float32)
    np.add.at(dcol, cols, adj_vals)
    inv_drow = np.where(drow > 0, 1.0 / drow, 0.0).astype(np.float32)
    inv_dcol = np.where(dcol > 0, 1.0 / dcol, 0.0).astype(np.float32)
    vals1 = (adj_vals * inv_drow[rows]).astype(np.float32)
    vals2 = (adj_vals * inv_dcol[cols]).astype(np.float32)
    m1, p1 = _shard_support(vals1, rows, cols)   # support1: gather rows, scatter cols
    m2, p2 = _shard_support(vals2, cols, rows)   # support2: gather cols, scatter rows
    return (m1, p1), (m2, p2)


def build_wtilde(weights):
    """W~ [5*D*BL, OUT*BL] bf16 with recurrence folded in (per batch group).
    Row r = m*(D*BL) + (d*BL + b'); col = o*BL + b'."""
    W = weights.reshape(D, 5, OUT)
    C = [W[:, 0] - W[:, 2], W[:, 1] - W[:, 4], 2.0 * W[:, 2], W[:, 3],
         2.0 * W[:, 4]]
    F = D * BL
    Wt = np.zeros((5 * F, OUT * BL), np.float32)
    for m in range(5):
        for d in range(D):
            for b in range(BL):
                Wt[m * F + d * BL + b, b::BL] = C[m][d]
    return Wt.astype(BF16)


def make_x0_full(inputs_f32):
    """[B, N, D] f32 -> [N, FEAT] bf16 with col = g*256 + d*4 + b'."""
    x = inputs_f32.transpose(1, 0, 2)            # [N, B, D]
    x = x.reshape(N, NGRP, BL, D)                # [N, g, b', d]
    x = x.transpose(0, 1, 3, 2)                  # [N, g, d, b']
    return np.ascontiguousarray(x.reshape(N, FEAT)).astype(BF16)


# ---------------------------------------------------------------- program

def build_program(meta1, meta2):
    nc = bacc.Bacc("TRN2", target_bir_lowering=False, debug=False,
                   num_devices=NCORES)
    dt = mybir.dt

    x0full = nc.dram_tensor("x0full", [N, FEAT], dt.bfloat16,
                            kind="ExternalInput")
    x0loc = nc.dram_tensor("x0loc", [SHARD, FEAT], dt.bfloat16,
                           kind="ExternalInput")
    iota_in = nc.dram_tensor("iota", [P, P], dt.bfloat16, kind="ExternalInput")
    wt_in = nc.dram_tensor("wt", [5 * D * BL, OB], dt.bfloat16,
                           kind="ExternalInput")
    bias_in = nc.dram_tensor("bias", [P, OB], dt.float32, kind="ExternalInput")

    sup_t = []
    for i, meta in enumerate((meta1, meta2)):
        ch = meta['ch']
        sup_t.append(dict(
            idx=nc.dram_tensor(f"idx{i}", [P, ch * 8], dt.int16,
                               kind="ExternalInput"),
            dst=nc.dram_tensor(f"dst{i}", [P, ch], dt.bfloat16,
                               kind="ExternalInput"),
            v=nc.dram_tensor(f"v{i}", [P, ch], dt.bfloat16,
                             kind="ExternalInput"),
        ))

    A1loc = nc.dram_tensor("A1loc", [SHARD, FEAT], dt.bfloat16, kind="Internal")
    B1loc = nc.dram_tensor("B1loc", [SHARD, FEAT], dt.bfloat16, kind="Internal")
    R2loc = nc.dram_tensor("R2loc", [SHARD, FEAT], dt.bfloat16, kind="Internal")
    R4loc = nc.dram_tensor("R4loc", [SHARD, FEAT], dt.bfloat16, kind="Internal")
    A1full = nc.dram_tensor("A1full", [N, FEAT], dt.bfloat16, kind="Internal",
                            addr_space="Shared")
    B1full = nc.dram_tensor("B1full", [N, FEAT], dt.bfloat16, kind="Internal",
                            addr_space="Shared")
    out = nc.dram_tensor("out", [SHARD, NGRP * OB], dt.float32,
                         kind="ExternalOutput")

    n_wchunks = 5 * D * BL // P  # 10

    with tile.TileContext(nc) as tc:
        with tc.tile_pool(name="const", bufs=1) as cpool:
            iota_sb = cpool.tile([P, P], dt.bfloat16)
            nc.sync.dma_start(iota_sb[:], iota_in[:, :])
            wt_sb = cpool.tile([P, n_wchunks, OB], dt.bfloat16)
            nc.sync.dma_start(
                wt_sb[:], wt_in[:, :].rearrange("(k p) o -> p k o", p=P))
            bias_sb = cpool.tile([P, OB], dt.float32)
            nc.sync.dma_start(bias_sb[:], bias_in[:, :])

            sup_sb = []
            for i, meta in enumerate((meta1, meta2)):
                ch = meta['ch']
                idx_sb = cpool.tile([P, ch * 8], dt.int16, name=f"idxsb{i}")
                nc.sync.dma_start(idx_sb[:], sup_t[i]['idx'][:, :])
                dst_sb = cpool.tile([P, ch], dt.bfloat16, name=f"dstsb{i}")
                nc.sync.dma_start(dst_sb[:], sup_t[i]['dst'][:, :])
                v_sb = cpool.tile([P, ch], dt.bfloat16, name=f"vsb{i}")
                nc.sync.dma_start(v_sb[:], sup_t[i]['v'][:, :])
                sup_sb.append((idx_sb, dst_sb, v_sb))

            def emit_spmm(meta, sbufs, table_ap, ydst, zpool, selpool, evpool,
                          pspool):
                idx_sb, dst_sb, v_sb = sbufs
                ch = meta['ch']
                cb = meta['chunk_block']
                first = meta['first']
                last = meta['last']
                ps = None
                for call in range(ch // CALL):
                    z = zpool.tile([P, CALL, FEAT], dt.bfloat16, tag="z")
                    nc.gpsimd.dma_gather(
                        z[:, :, :], table_ap,
                        idx_sb[:, call * (CALL * 8):(call + 1) * (CALL * 8)],
                        CALL * P, CALL * P, FEAT, single_packet=False)
                    sel = selpool.tile([P, CALL, P], dt.bfloat16, tag="sel")
                    for g0 in range(0, CALL, SELG):
                        c0 = call * CALL + g0
                        sl = sel[:, g0:g0 + SELG, :]
                        nc.vector.tensor_tensor(
                            out=sl,
                            in0=iota_sb[:][:, None, :].to_broadcast(
                                [P, SELG, P]),
                            in1=dst_sb[:, c0:c0 + SELG, None].to_broadcast(
                                [P, SELG, P]),
                            op=mybir.AluOpType.is_equal)
                        nc.vector.tensor_tensor(
                            out=sl,
                            in0=sl,
                            in1=v_sb[:, c0:c0 + SELG, None].to_broadcast(
                                [P, SELG, P]),
                            op=mybir.AluOpType.mult)
                    for cl in range(CALL):
                        c = call * CALL + cl
                        if first[c]:
                            ps = [pspool.tile([P, SEC], dt.float32,
                                              tag=f"s{k}", name=f"ps{k}")
                                  for k in range(NSEC)]
                        for k in range(NSEC):
                            nc.tensor.matmul(
                                out=ps[k][:],
                                lhsT=sel[:, cl, :],
                                rhs=z[:, cl, k * SEC:(k + 1) * SEC],
                                start=bool(first[c]),
                                stop=bool(last[c]),
                            )
                        if last[c]:
                            b = cb[c]
                            y = evpool.tile([P, FEAT], dt.bfloat16, tag="y")
                            for k in range(NSEC):
                                nc.vector.tensor_copy(
                                    out=y[:, k * SEC:(k + 1) * SEC],
                                    in_=ps[k][:])
                            nc.sync.dma_start(
                                ydst[b * P:(b + 1) * P, :], y[:])

            with (
                tc.tile_pool(name="z", bufs=2) as zpool,
                tc.tile_pool(name="sel", bufs=2) as selpool,
                tc.tile_pool(name="ev", bufs=2) as evpool,
                tc.tile_pool(name="ps", bufs=2, space="PSUM") as pspool,
            ):
                pools = (zpool, selpool, evpool, pspool)
                rg = [list(range(NCORES))]

                emit_spmm(meta1, sup_sb[0], x0full[:, :], A1loc, *pools)
                nc.gpsimd.collective_compute(
                    "AllGather", mybir.AluOpType.bypass, replica_groups=rg,
                    ins=[A1loc[:, :]], outs=[A1full[:, :]])
                emit_spmm(meta2, sup_sb[1], A1full[:, :], B1loc, *pools)
                emit_spmm(meta1, sup_sb[0], A1full[:, :], R2loc, *pools)
                nc.gpsimd.collective_compute(
                    "AllGather", mybir.AluOpType.bypass, replica_groups=rg,
                    ins=[B1loc[:, :]], outs=[B1full[:, :]])
                emit_spmm(meta2, sup_sb[1], B1full[:, :], R4loc, *pools)

            # ---------------- projection
            xs = [x0loc, A1loc, R2loc, B1loc, R4loc]
            with (
                tc.tile_pool(name="lhs", bufs=4) as lpool,
                tc.tile_pool(name="po", bufs=2) as opool,
                tc.tile_pool(name="pso", bufs=2, space="PSUM") as psopool,
            ):
                for t in range(NBLK):
                    rows = slice(t * P, (t + 1) * P)
                    for g in range(NGRP):
                        pso = psopool.tile([P, OB], dt.float32, tag="pso")
                        for k in range(n_wchunks):
                            m, h = divmod(k, 2)
                            lhsT = lpool.tile([P, P], dt.bfloat16, tag="lhsT")
                            nc.sync.dma_start_transpose(
                                lhsT[:],
                                xs[m][rows,
                                      g * OB + h * P:g * OB + (h + 1) * P])
                            nc.tensor.matmul(
                                out=pso[:],
                                lhsT=lhsT[:],
                                rhs=wt_sb[:, k, :],
                                start=(k == 0),
                                stop=(k == n_wchunks - 1),
                            )
                        osb = opool.tile([P, OB], dt.float32, tag="osb")
                        nc.vector.tensor_tensor(
                            out=osb[:], in0=pso[:], in1=bias_sb[:],
                            op=mybir.AluOpType.add)
                        nc.sync.dma_start(out[rows, g * OB:(g + 1) * OB],
                                          osb[:])

    nc.compile()
    return nc


# ---------------------------------------------------------------- entry

def prepare(inputs):
    """Host prep: returns (nc, in_maps)."""
    inputs_f32 = np.asarray(inputs['inputs'], dtype=np.float32)
    adj_vals = np.asarray(inputs['adj_vals'], dtype=np.float32)
    rows = np.asarray(inputs['rows']).astype(np.int64)
    cols = np.asarray(inputs['cols']).astype(np.int64)
    weights = np.asarray(inputs['weights'], dtype=np.float32)
    biases = np.asarray(inputs['biases'], dtype=np.float32)

    (m1, p1), (m2, p2) = preprocess(adj_vals, rows, cols)
    wt = build_wtilde(weights)
    x0 = make_x0_full(inputs_f32)
    iota = np.tile(np.arange(P, dtype=np.float32)[None, :], (P, 1)).astype(BF16)
    bias_rep = np.zeros((P, OB), np.float32)
    for o in range(OUT):
        bias_rep[:, o * BL:(o + 1) * BL] = biases[0, o]

    nc = build_program(m1, m2)

    in_maps = []
    for c in range(NCORES):
        in_maps.append(dict(
            x0full=x0, x0loc=np.ascontiguousarray(
                x0[c * SHARD:(c + 1) * SHARD]),
            iota=iota, wt=wt, bias=bias_rep,
            idx0=p1[c]['idx_w'], dst0=p1[c]['dst_t'], v0=p1[c]['v_t'],
            idx1=p2[c]['idx_w'], dst1=p2[c]['dst_t'], v1=p2[c]['v_t'],
        ))
    return nc, in_maps


def postprocess(results):
    """Per-core out [SHARD, NGRP*OB] f32 (col = g*256 + o*4 + b') ->
    full [B, N, OUT] f32."""
    out = np.zeros((BTOT, N, OUT), np.float32)
    for c in range(NCORES):
        oc = results[c]['out']
        v = oc.reshape(SHARD, NGRP, OUT, BL)      # [n, g, o, b']
        v = v.transpose(1, 3, 0, 2)               # [g, b', n, o]
        out[:, c * SHARD:(c + 1) * SHARD] = v.reshape(BTOT, SHARD, OUT)
    return out


def kernel(**inputs):
    nc, in_maps = prepare(inputs)
    res = run_bass_kernel_spmd(nc, in_maps, core_ids=list(range(NCORES)))
    return postprocess(res.results)


# revision 12
# speedup vs baseline: 3.9985x; 1.2836x over previous
"""Trainium2 Bass kernel for nn_DConv (diffusion graph conv, K=2, 2 supports).

Contract: kernel(**inputs) takes FULL unsharded inputs (inputs [B,N,D] f32,
adj_vals [E] f32, rows/cols [E] int, weights [D*M,OUT] f32, biases [1,OUT]
f32) and returns the FULL output [B, N, OUT] f32.

Strategy (v1: dst-node-sharded edges + AllGather, all batches per core):
 - The bottleneck in the batch-parallel design was GpSimd (Q7) SWDGE
   descriptor generation for dma_gather at ~7.6ns/index: each core gathered
   all E=524288 edges per spmm. Here each core owns 1/8 of the DESTINATION
   nodes (rows [2048c, 2048(c+1))) and processes only the ~E/8 edges that
   land there, but carries ALL 32 batches (row width 64*32 = 2048 bf16 =
   4KB). Same DMA bytes per core, 8x fewer gather descriptors.
 - x tables are full [16384, 2048] bf16 (64MB). Column order: col =
   g*256 + d*4 + b' where b = 4g + b' (8 batch-groups of 4) so the
   projection reuses the baseline 256-col block-diagonal W~ per group.
 - Per spmm: sort own edges by dst into 16 local 128-node blocks, pad each
   block segment to a UNIFORM (across cores) number of 128-slot chunks so
   one SPMD program fits all cores. dma_gather (4KB rows) fetches x[src]
   per slot; DVE builds Sel[e, dst_local] = (iota==dst)*v in bf16; TensorE
   scatters chunk into 4 PSUM sections [128,512] f32 per dst block.
 - Chebyshev recurrence folded into projection weights (host): raw products
   A1 = S1 X0, B1 = S2 A1, R2 = S1 A1, R4 = S2 B1;
   out = X0(W0-W2) + A1(W1-W4) + R2(2 W2) + B1 W3 + R4(2 W4) + bias.
 - Two AllGathers move A1 and B1 shards (8MB each) to full tables (the only
   cross-spmm tables that are gathered from). R2/R4 stay local. B1 is
   emitted before R2 so AllGather(B1) overlaps R2's gathers.
 - Projection: per node tile (16) x batch group (8): DMA-transpose loads
   x_m^T col-slices, TensorE contracts against block-diag W~ [1280, 256].
"""
import os
import sys
import numpy as np
import ml_dtypes

for _p in ('/opt/trn_rl_repo', '/root/.axon_site/_ro/trn_rl_repo'):
    if os.path.isdir(_p) and _p not in sys.path:
        sys.path.append(_p)

import concourse.bass as bass
import concourse.mybir as mybir
import concourse.tile as tile
from concourse import bacc
from concourse.bass_utils import run_bass_kernel_spmd

BF16 = ml_dtypes.bfloat16
P = 128
NCORES = 8
N = 16384
SHARD = N // NCORES          # 2048 nodes per core
NBLK = SHARD // P            # 16 local dst blocks per core
D = 64
BTOT = 32
BL = 4                       # batches per projection group
NGRP = BTOT // BL            # 8 groups
FEAT = D * BTOT              # 2048 columns (4KB bf16 rows)
OUT = 64
OB = OUT * BL                # 256 out cols per group
CALL = 16                    # chunks per dma_gather call (2048 indices)
SELG = 8                     # chunks per DVE sel-build op
NSEC = 4                     # PSUM sections per block (4 x [128,512] f32)
SEC = FEAT // NSEC           # 512
TT = 4                       # node tiles per projection super-tile


# ---------------------------------------------------------------- host prep

def _shard_support(vals, src, dst):
    """Partition edges by dst block range of 2048 nodes per core; sort each
    core's edges by dst; pad each local 128-node block segment to a number of
    chunks that is UNIFORM across cores (program structure must be SPMD).

    Returns (meta, percore) where meta has the uniform chunk structure and
    percore[c] has idx_w/dst_t/v_t slot arrays."""
    per_core_sorted = []
    cnt = np.zeros((NCORES, NBLK), np.int64)
    for c in range(NCORES):
        lo, hi = SHARD * c, SHARD * (c + 1)
        m = (dst >= lo) & (dst < hi)
        s_dst = dst[m] - lo
        s_src = src[m]
        s_v = vals[m]
        order = np.argsort(s_dst, kind='stable')
        s_dst = s_dst[order]
        s_src = s_src[order]
        s_v = s_v[order]
        blk = s_dst // P
        cnt[c] = np.bincount(blk, minlength=NBLK)
        per_core_sorted.append((s_src, s_dst, s_v))

    cpb = np.maximum(1, -(-cnt.max(axis=0) // P))  # chunks per block (uniform)
    ch = int(cpb.sum())
    pad = (-ch) % CALL
    cpb[NBLK - 1] += pad
    ch += pad

    chunk_block = np.repeat(np.arange(NBLK), cpb)
    first = np.ones(ch, bool)
    first[1:] = chunk_block[1:] != chunk_block[:-1]
    last = np.ones(ch, bool)
    last[:-1] = chunk_block[:-1] != chunk_block[1:]
    meta = dict(ch=ch, cpb=cpb, chunk_block=chunk_block, first=first, last=last)

    percore = []
    for c in range(NCORES):
        s_src, s_dst, s_v = per_core_sorted[c]
        n_slots = ch * P
        slot_src = np.zeros(n_slots, np.int16)
        slot_dstl = np.zeros(n_slots, np.float32)
        slot_v = np.zeros(n_slots, np.float32)
        pos = 0
        off = 0
        for b in range(NBLK):
            n = int(cnt[c, b])
            sl = slice(off, off + n)
            slot_src[sl] = s_src[pos:pos + n].astype(np.int16)
            slot_dstl[sl] = (s_dst[pos:pos + n] - b * P).astype(np.float32)
            slot_v[sl] = s_v[pos:pos + n]
            pos += n
            off += int(cpb[b]) * P
        # wrapped idx layout [128, n_slots/16]: col j = slots [16j, 16j+16)
        idx_w = np.ascontiguousarray(
            np.tile(slot_src.reshape(-1, 16).T, (8, 1)))
        dst_t = np.ascontiguousarray(
            slot_dstl.reshape(ch, P).T).astype(BF16)
        v_t = np.ascontiguousarray(slot_v.reshape(ch, P).T).astype(BF16)
        percore.append(dict(idx_w=idx_w, dst_t=dst_t, v_t=v_t))
    return meta, percore


def preprocess(adj_vals, rows, cols):
    drow = np.zeros(N, np.float32)
    np.add.at(drow, rows, adj_vals)
    dcol = np.zeros(N, np.float32)
    np.add.at(dcol, cols, adj_vals)
    inv_drow = np.where(drow > 0, 1.0 / drow, 0.0).astype(np.float32)
    inv_dcol = np.where(dcol > 0, 1.0 / dcol, 0.0).astype(np.float32)
    vals1 = (adj_vals * inv_drow[rows]).astype(np.float32)
    vals2 = (adj_vals * inv_dcol[cols]).astype(np.float32)
    m1, p1 = _shard_support(vals1, rows, cols)   # support1: gather rows, scatter cols
    m2, p2 = _shard_support(vals2, cols, rows)   # support2: gather cols, scatter rows
    return (m1, p1), (m2, p2)


def build_wtilde(weights):
    """W~ [5*D*BL, OUT*BL] bf16 with recurrence folded in (per batch group).
    Row r = m*(D*BL) + (d*BL + b'); col = o*BL + b'."""
    W = weights.reshape(D, 5, OUT)
    C = [W[:, 0] - W[:, 2], W[:, 1] - W[:, 4], 2.0 * W[:, 2], W[:, 3],
         2.0 * W[:, 4]]
    F = D * BL
    Wt = np.zeros((5 * F, OUT * BL), np.float32)
    for m in range(5):
        for d in range(D):
            for b in range(BL):
                Wt[m * F + d * BL + b, b::BL] = C[m][d]
    return Wt.astype(BF16)


def make_x0_full(inputs_f32):
    """[B, N, D] f32 -> [N, FEAT] bf16 with col = g*256 + d*4 + b'."""
    x = inputs_f32.transpose(1, 0, 2)            # [N, B, D]
    x = x.reshape(N, NGRP, BL, D)                # [N, g, b', d]
    x = x.transpose(0, 1, 3, 2)                  # [N, g, d, b']
    return np.ascontiguousarray(x.reshape(N, FEAT)).astype(BF16)


# ---------------------------------------------------------------- program

def build_program(meta1, meta2):
    nc = bacc.Bacc("TRN2", target_bir_lowering=False, debug=False,
                   num_devices=NCORES)
    dt = mybir.dt

    x0full = nc.dram_tensor("x0full", [N, FEAT], dt.bfloat16,
                            kind="ExternalInput")
    x0loc = nc.dram_tensor("x0loc", [SHARD, FEAT], dt.bfloat16,
                           kind="ExternalInput")
    iota_in = nc.dram_tensor("iota", [P, P], dt.bfloat16, kind="ExternalInput")
    wt_in = nc.dram_tensor("wt", [5 * D * BL, OB], dt.bfloat16,
                           kind="ExternalInput")
    bias_in = nc.dram_tensor("bias", [P, OB], dt.float32, kind="ExternalInput")

    sup_t = []
    for i, meta in enumerate((meta1, meta2)):
        ch = meta['ch']
        sup_t.append(dict(
            idx=nc.dram_tensor(f"idx{i}", [P, ch * 8], dt.int16,
                               kind="ExternalInput"),
            dst=nc.dram_tensor(f"dst{i}", [P, ch], dt.bfloat16,
                               kind="ExternalInput"),
            v=nc.dram_tensor(f"v{i}", [P, ch], dt.bfloat16,
                             kind="ExternalInput"),
        ))

    A1loc = nc.dram_tensor("A1loc", [SHARD, FEAT], dt.bfloat16, kind="Internal")
    B1loc = nc.dram_tensor("B1loc", [SHARD, FEAT], dt.bfloat16, kind="Internal")
    R2loc = nc.dram_tensor("R2loc", [SHARD, FEAT], dt.bfloat16, kind="Internal")
    R4loc = nc.dram_tensor("R4loc", [SHARD, FEAT], dt.bfloat16, kind="Internal")
    A1full = nc.dram_tensor("A1full", [N, FEAT], dt.bfloat16, kind="Internal",
                            addr_space="Shared")
    B1full = nc.dram_tensor("B1full", [N, FEAT], dt.bfloat16, kind="Internal",
                            addr_space="Shared")
    partial = nc.dram_tensor("partial", [SHARD, NGRP * OB], dt.float32,
                             kind="Internal")
    out = nc.dram_tensor("out", [SHARD, NGRP * OB], dt.float32,
                         kind="ExternalOutput")

    n_wchunks = 5 * D * BL // P  # 10

    with tile.TileContext(nc) as tc:
        with tc.tile_pool(name="const", bufs=1) as cpool:
            iota_sb = cpool.tile([P, P], dt.bfloat16)
            nc.sync.dma_start(iota_sb[:], iota_in[:, :])
            wt_sb = cpool.tile([P, n_wchunks, OB], dt.bfloat16)
            nc.sync.dma_start(
                wt_sb[:], wt_in[:, :].rearrange("(k p) o -> p k o", p=P))
            bias_sb = cpool.tile([P, OB], dt.float32)
            nc.sync.dma_start(bias_sb[:], bias_in[:, :])

            sup_sb = []
            for i, meta in enumerate((meta1, meta2)):
                ch = meta['ch']
                idx_sb = cpool.tile([P, ch * 8], dt.int16, name=f"idxsb{i}")
                nc.sync.dma_start(idx_sb[:], sup_t[i]['idx'][:, :])
                dst_sb = cpool.tile([P, ch], dt.bfloat16, name=f"dstsb{i}")
                nc.sync.dma_start(dst_sb[:], sup_t[i]['dst'][:, :])
                v_sb = cpool.tile([P, ch], dt.bfloat16, name=f"vsb{i}")
                nc.sync.dma_start(v_sb[:], sup_t[i]['v'][:, :])
                sup_sb.append((idx_sb, dst_sb, v_sb))

            xs = [x0loc, A1loc, R2loc, B1loc, R4loc]

            def emit_spmm(meta, sbufs, table_ap, ydst, zpool, selpool, evpool,
                          pspool, interleave=None):
                idx_sb, dst_sb, v_sb = sbufs
                ch = meta['ch']
                cb = meta['chunk_block']
                first = meta['first']
                last = meta['last']
                ps = None
                for call in range(ch // CALL):
                    z = zpool.tile([P, CALL, FEAT], dt.bfloat16, tag="z")
                    nc.gpsimd.dma_gather(
                        z[:, :, :], table_ap,
                        idx_sb[:, call * (CALL * 8):(call + 1) * (CALL * 8)],
                        CALL * P, CALL * P, FEAT, single_packet=False)
                    sel = selpool.tile([P, CALL, P], dt.bfloat16, tag="sel")
                    for g0 in range(0, CALL, SELG):
                        c0 = call * CALL + g0
                        sl = sel[:, g0:g0 + SELG, :]
                        nc.vector.tensor_tensor(
                            out=sl,
                            in0=iota_sb[:][:, None, :].to_broadcast(
                                [P, SELG, P]),
                            in1=dst_sb[:, c0:c0 + SELG, None].to_broadcast(
                                [P, SELG, P]),
                            op=mybir.AluOpType.is_equal)
                        nc.vector.tensor_tensor(
                            out=sl,
                            in0=sl,
                            in1=v_sb[:, c0:c0 + SELG, None].to_broadcast(
                                [P, SELG, P]),
                            op=mybir.AluOpType.mult)
                    for cl in range(CALL):
                        c = call * CALL + cl
                        if first[c]:
                            ps = [pspool.tile([P, SEC], dt.float32,
                                              tag=f"s{k}", name=f"ps{k}")
                                  for k in range(NSEC)]
                        for k in range(NSEC):
                            nc.tensor.matmul(
                                out=ps[k][:],
                                lhsT=sel[:, cl, :],
                                rhs=z[:, cl, k * SEC:(k + 1) * SEC],
                                start=bool(first[c]),
                                stop=bool(last[c]),
                            )
                        if last[c]:
                            b = cb[c]
                            y = evpool.tile([P, FEAT], dt.bfloat16, tag="y")
                            for k in range(NSEC):
                                nc.vector.tensor_copy(
                                    out=y[:, k * SEC:(k + 1) * SEC],
                                    in_=ps[k][:])
                            nc.sync.dma_start(
                                ydst[b * P:(b + 1) * P, :], y[:])
                    if interleave is not None:
                        interleave(call)

            def emit_proj_unit(t8, g, ks, pass2, lpool, psopool, opool,
                               ppool):
                rows = slice(t8 * TT * P, (t8 + 1) * TT * P)
                pso_banks = [psopool.tile([P, 2 * OB], dt.float32,
                                          tag=f"psop{i}", name=f"psop{i}")
                             for i in range(TT)]
                pso = [pso_banks[tt][:, :OB] for tt in range(TT)]
                for j, k in enumerate(ks):
                    m, h = divmod(k, 2)
                    lt = lpool.tile([P, TT * P], dt.bfloat16, tag="lt")
                    nc.sync.dma_start_transpose(
                        lt[:],
                        xs[m][rows, g * OB + h * P:g * OB + (h + 1) * P])
                    for tt in range(TT):
                        nc.tensor.matmul(
                            out=pso[tt],
                            lhsT=lt[:, tt * P:(tt + 1) * P],
                            rhs=wt_sb[:, k, :],
                            start=(j == 0),
                            stop=(j == len(ks) - 1),
                        )
                for tt in range(TT):
                    r2 = slice((t8 * TT + tt) * P, (t8 * TT + tt + 1) * P)
                    osb = opool.tile([P, OB], dt.float32, tag="osb")
                    if pass2:
                        par = ppool.tile([P, OB], dt.float32, tag="par")
                        nc.sync.dma_start(par[:],
                                          partial[r2, g * OB:(g + 1) * OB])
                        nc.vector.tensor_tensor(
                            out=osb[:], in0=pso[tt], in1=par[:],
                            op=mybir.AluOpType.add)
                        nc.sync.dma_start(out[r2, g * OB:(g + 1) * OB],
                                          osb[:])
                    else:
                        nc.vector.tensor_tensor(
                            out=osb[:], in0=pso[tt], in1=bias_sb[:],
                            op=mybir.AluOpType.add)
                        nc.sync.dma_start(partial[r2, g * OB:(g + 1) * OB],
                                          osb[:])

            rg = [list(range(NCORES))]
            with (
                tc.tile_pool(name="z", bufs=2) as zpool,
                tc.tile_pool(name="sel", bufs=2) as selpool,
                tc.tile_pool(name="ev", bufs=2) as evpool,
            ):
                with tc.tile_pool(name="ps", bufs=2, space="PSUM") as pspool:
                    pools = (zpool, selpool, evpool, pspool)
                    emit_spmm(meta1, sup_sb[0], x0full[:, :], A1loc, *pools)
                    nc.gpsimd.collective_compute(
                        "AllGather", mybir.AluOpType.bypass,
                        replica_groups=rg,
                        ins=[A1loc[:, :]], outs=[A1full[:, :]])
                    emit_spmm(meta2, sup_sb[1], A1full[:, :], B1loc, *pools)
                    nc.gpsimd.collective_compute(
                        "AllGather", mybir.AluOpType.bypass,
                        replica_groups=rg,
                        ins=[B1loc[:, :]], outs=[B1full[:, :]])
                    emit_spmm(meta1, sup_sb[0], A1full[:, :], R2loc, *pools)

                with (
                    tc.tile_pool(name="ps2", bufs=1, space="PSUM") as pspool2,
                    tc.tile_pool(name="pso", bufs=1, space="PSUM") as psopool,
                    tc.tile_pool(name="lhs", bufs=3) as lpool,
                    tc.tile_pool(name="po", bufs=2) as opool,
                    tc.tile_pool(name="par", bufs=4) as ppool,
                ):
                    units = [(t8, g) for t8 in range(SHARD // (TT * P))
                             for g in range(NGRP)]
                    uidx = [0]

                    def interleave(call):
                        if call % 2 == 1 and uidx[0] < len(units):
                            t8, g = units[uidx[0]]
                            uidx[0] += 1
                            emit_proj_unit(t8, g, range(6), False,
                                           lpool, psopool, opool, ppool)

                    emit_spmm(meta2, sup_sb[1], B1full[:, :], R4loc,
                              zpool, selpool, evpool, pspool2, interleave)
                    while uidx[0] < len(units):
                        t8, g = units[uidx[0]]
                        uidx[0] += 1
                        emit_proj_unit(t8, g, range(6), False,
                                       lpool, psopool, opool, ppool)
                    for t8, g in units:
                        emit_proj_unit(t8, g, range(6, n_wchunks), True,
                                       lpool, psopool, opool, ppool)

    nc.compile()
    return nc


# ---------------------------------------------------------------- entry

def prepare(inputs):
    """Host prep: returns (nc, in_maps)."""
    inputs_f32 = np.asarray(inputs['inputs'], dtype=np.float32)
    adj_vals = np.asarray(inputs['adj_vals'], dtype=np.float32)
    rows = np.asarray(inputs['rows']).astype(np.int64)
    cols = np.asarray(inputs['cols']).astype(np.int64)
    weights = np.asarray(inputs['weights'], dtype=np.float32)
    biases = np.asarray(inputs['biases'], dtype=np.float32)

    (m1, p1), (m2, p2) = preprocess(adj_vals, rows, cols)
    wt = build_wtilde(weights)
    x0 = make_x0_full(inputs_f32)
    iota = np.tile(np.arange(P, dtype=np.float32)[None, :], (P, 1)).astype(BF16)
    bias_rep = np.zeros((P, OB), np.float32)
    for o in range(OUT):
        bias_rep[:, o * BL:(o + 1) * BL] = biases[0, o]

    nc = build_program(m1, m2)

    in_maps = []
    for c in range(NCORES):
        in_maps.append(dict(
            x0full=x0, x0loc=np.ascontiguousarray(
                x0[c * SHARD:(c + 1) * SHARD]),
            iota=iota, wt=wt, bias=bias_rep,
            idx0=p1[c]['idx_w'], dst0=p1[c]['dst_t'], v0=p1[c]['v_t'],
            idx1=p2[c]['idx_w'], dst1=p2[c]['dst_t'], v1=p2[c]['v_t'],
        ))
    return nc, in_maps


def postprocess(results):
    """Per-core out [SHARD, NGRP*OB] f32 (col = g*256 + o*4 + b') ->
    full [B, N, OUT] f32."""
    out = np.zeros((BTOT, N, OUT), np.float32)
    for c in range(NCORES):
        oc = results[c]['out']
        v = oc.reshape(SHARD, NGRP, OUT, BL)      # [n, g, o, b']
        v = v.transpose(1, 3, 0, 2)               # [g, b', n, o]
        out[:, c * SHARD:(c + 1) * SHARD] = v.reshape(BTOT, SHARD, OUT)
    return out


def kernel(**inputs):
    nc, in_maps = prepare(inputs)
    res = run_bass_kernel_spmd(nc, in_maps, core_ids=list(range(NCORES)))
    return postprocess(res.results)


# revision 13
# speedup vs baseline: 4.1185x; 1.0300x over previous
"""Trainium2 Bass kernel for nn_DConv (diffusion graph conv, K=2, 2 supports).

Contract: kernel(**inputs) takes FULL unsharded inputs (inputs [B,N,D] f32,
adj_vals [E] f32, rows/cols [E] int, weights [D*M,OUT] f32, biases [1,OUT]
f32) and returns the FULL output [B, N, OUT] f32.

Strategy (v1: dst-node-sharded edges + AllGather, all batches per core):
 - The bottleneck in the batch-parallel design was GpSimd (Q7) SWDGE
   descriptor generation for dma_gather at ~7.6ns/index: each core gathered
   all E=524288 edges per spmm. Here each core owns 1/8 of the DESTINATION
   nodes (rows [2048c, 2048(c+1))) and processes only the ~E/8 edges that
   land there, but carries ALL 32 batches (row width 64*32 = 2048 bf16 =
   4KB). Same DMA bytes per core, 8x fewer gather descriptors.
 - x tables are full [16384, 2048] bf16 (64MB). Column order: col =
   g*256 + d*4 + b' where b = 4g + b' (8 batch-groups of 4) so the
   projection reuses the baseline 256-col block-diagonal W~ per group.
 - Per spmm: sort own edges by dst into 16 local 128-node blocks, pad each
   block segment to a UNIFORM (across cores) number of 128-slot chunks so
   one SPMD program fits all cores. dma_gather (4KB rows) fetches x[src]
   per slot; DVE builds Sel[e, dst_local] = (iota==dst)*v in bf16; TensorE
   scatters chunk into 4 PSUM sections [128,512] f32 per dst block.
 - Chebyshev recurrence folded into projection weights (host): raw products
   A1 = S1 X0, B1 = S2 A1, R2 = S1 A1, R4 = S2 B1;
   out = X0(W0-W2) + A1(W1-W4) + R2(2 W2) + B1 W3 + R4(2 W4) + bias.
 - Two AllGathers move A1 and B1 shards (8MB each) to full tables (the only
   cross-spmm tables that are gathered from). R2/R4 stay local. B1 is
   emitted before R2 so AllGather(B1) overlaps R2's gathers.
 - Projection: per node tile (16) x batch group (8): DMA-transpose loads
   x_m^T col-slices, TensorE contracts against block-diag W~ [1280, 256].
"""
import os
import sys
import numpy as np
import ml_dtypes

for _p in ('/opt/trn_rl_repo', '/root/.axon_site/_ro/trn_rl_repo'):
    if os.path.isdir(_p) and _p not in sys.path:
        sys.path.append(_p)

import concourse.bass as bass
import concourse.mybir as mybir
import concourse.tile as tile
from concourse import bacc
from concourse.bass_utils import run_bass_kernel_spmd

BF16 = ml_dtypes.bfloat16
P = 128
NCORES = 8
N = 16384
SHARD = N // NCORES          # 2048 nodes per core
NBLK = SHARD // P            # 16 local dst blocks per core
D = 64
BTOT = 32
BL = 4                       # batches per projection group
NGRP = BTOT // BL            # 8 groups
FEAT = D * BTOT              # 2048 columns (4KB bf16 rows)
OUT = 64
OB = OUT * BL                # 256 out cols per group
CALL = 8                     # chunks per dma_gather call (1024 indices)
SELG = 8                     # chunks per DVE sel-build op
NSEC = 4                     # PSUM sections per block (4 x [128,512] f32)
SEC = FEAT // NSEC           # 512
TT = 4                       # node tiles per projection super-tile


# ---------------------------------------------------------------- host prep

def _shard_support(vals, src, dst):
    """Partition edges by dst block range of 2048 nodes per core; sort each
    core's edges by dst; pad each local 128-node block segment to a number of
    chunks that is UNIFORM across cores (program structure must be SPMD).

    Returns (meta, percore) where meta has the uniform chunk structure and
    percore[c] has idx_w/dst_t/v_t slot arrays."""
    per_core_sorted = []
    cnt = np.zeros((NCORES, NBLK), np.int64)
    for c in range(NCORES):
        lo, hi = SHARD * c, SHARD * (c + 1)
        m = (dst >= lo) & (dst < hi)
        s_dst = dst[m] - lo
        s_src = src[m]
        s_v = vals[m]
        order = np.argsort(s_dst, kind='stable')
        s_dst = s_dst[order]
        s_src = s_src[order]
        s_v = s_v[order]
        blk = s_dst // P
        cnt[c] = np.bincount(blk, minlength=NBLK)
        per_core_sorted.append((s_src, s_dst, s_v))

    cpb = np.maximum(1, -(-cnt.max(axis=0) // P))  # chunks per block (uniform)
    ch = int(cpb.sum())
    pad = (-ch) % CALL
    cpb[NBLK - 1] += pad
    ch += pad

    chunk_block = np.repeat(np.arange(NBLK), cpb)
    first = np.ones(ch, bool)
    first[1:] = chunk_block[1:] != chunk_block[:-1]
    last = np.ones(ch, bool)
    last[:-1] = chunk_block[:-1] != chunk_block[1:]
    meta = dict(ch=ch, cpb=cpb, chunk_block=chunk_block, first=first, last=last)

    percore = []
    for c in range(NCORES):
        s_src, s_dst, s_v = per_core_sorted[c]
        n_slots = ch * P
        slot_src = np.zeros(n_slots, np.int16)
        slot_dstl = np.zeros(n_slots, np.float32)
        slot_v = np.zeros(n_slots, np.float32)
        pos = 0
        off = 0
        for b in range(NBLK):
            n = int(cnt[c, b])
            sl = slice(off, off + n)
            slot_src[sl] = s_src[pos:pos + n].astype(np.int16)
            slot_dstl[sl] = (s_dst[pos:pos + n] - b * P).astype(np.float32)
            slot_v[sl] = s_v[pos:pos + n]
            pos += n
            off += int(cpb[b]) * P
        # wrapped idx layout [128, n_slots/16]: col j = slots [16j, 16j+16)
        idx_w = np.ascontiguousarray(
            np.tile(slot_src.reshape(-1, 16).T, (8, 1)))
        dst_t = np.ascontiguousarray(
            slot_dstl.reshape(ch, P).T).astype(BF16)
        v_t = np.ascontiguousarray(slot_v.reshape(ch, P).T).astype(BF16)
        percore.append(dict(idx_w=idx_w, dst_t=dst_t, v_t=v_t))
    return meta, percore


def preprocess(adj_vals, rows, cols):
    drow = np.zeros(N, np.float32)
    np.add.at(drow, rows, adj_vals)
    dcol = np.zeros(N, np.float32)
    np.add.at(dcol, cols, adj_vals)
    inv_drow = np.where(drow > 0, 1.0 / drow, 0.0).astype(np.float32)
    inv_dcol = np.where(dcol > 0, 1.0 / dcol, 0.0).astype(np.float32)
    vals1 = (adj_vals * inv_drow[rows]).astype(np.float32)
    vals2 = (adj_vals * inv_dcol[cols]).astype(np.float32)
    m1, p1 = _shard_support(vals1, rows, cols)   # support1: gather rows, scatter cols
    m2, p2 = _shard_support(vals2, cols, rows)   # support2: gather cols, scatter rows
    return (m1, p1), (m2, p2)


def build_wtilde(weights):
    """W~ [5*D*BL, OUT*BL] bf16 with recurrence folded in (per batch group).
    Row r = m*(D*BL) + (d*BL + b'); col = o*BL + b'."""
    W = weights.reshape(D, 5, OUT)
    C = [W[:, 0] - W[:, 2], W[:, 1] - W[:, 4], 2.0 * W[:, 2], W[:, 3],
         2.0 * W[:, 4]]
    F = D * BL
    Wt = np.zeros((5 * F, OUT * BL), np.float32)
    for m in range(5):
        for d in range(D):
            for b in range(BL):
                Wt[m * F + d * BL + b, b::BL] = C[m][d]
    return Wt.astype(BF16)


def make_x0_full(inputs_f32):
    """[B, N, D] f32 -> [N, FEAT] bf16 with col = g*256 + d*4 + b'."""
    x = inputs_f32.transpose(1, 0, 2)            # [N, B, D]
    x = x.reshape(N, NGRP, BL, D)                # [N, g, b', d]
    x = x.transpose(0, 1, 3, 2)                  # [N, g, d, b']
    return np.ascontiguousarray(x.reshape(N, FEAT)).astype(BF16)


# ---------------------------------------------------------------- program

def build_program(meta1, meta2):
    nc = bacc.Bacc("TRN2", target_bir_lowering=False, debug=False,
                   num_devices=NCORES)
    dt = mybir.dt

    x0full = nc.dram_tensor("x0full", [N, FEAT], dt.bfloat16,
                            kind="ExternalInput")
    x0loc = nc.dram_tensor("x0loc", [SHARD, FEAT], dt.bfloat16,
                           kind="ExternalInput")
    iota_in = nc.dram_tensor("iota", [P, P], dt.bfloat16, kind="ExternalInput")
    wt_in = nc.dram_tensor("wt", [5 * D * BL, OB], dt.bfloat16,
                           kind="ExternalInput")
    bias_in = nc.dram_tensor("bias", [P, OB], dt.float32, kind="ExternalInput")

    sup_t = []
    for i, meta in enumerate((meta1, meta2)):
        ch = meta['ch']
        sup_t.append(dict(
            idx=nc.dram_tensor(f"idx{i}", [P, ch * 8], dt.int16,
                               kind="ExternalInput"),
            dst=nc.dram_tensor(f"dst{i}", [P, ch], dt.bfloat16,
                               kind="ExternalInput"),
            v=nc.dram_tensor(f"v{i}", [P, ch], dt.bfloat16,
                             kind="ExternalInput"),
        ))

    A1loc = nc.dram_tensor("A1loc", [SHARD, FEAT], dt.bfloat16, kind="Internal")
    B1loc = nc.dram_tensor("B1loc", [SHARD, FEAT], dt.bfloat16, kind="Internal")
    R2loc = nc.dram_tensor("R2loc", [SHARD, FEAT], dt.bfloat16, kind="Internal")
    R4loc = nc.dram_tensor("R4loc", [SHARD, FEAT], dt.bfloat16, kind="Internal")
    A1full = nc.dram_tensor("A1full", [N, FEAT], dt.bfloat16, kind="Internal",
                            addr_space="Shared")
    B1full = nc.dram_tensor("B1full", [N, FEAT], dt.bfloat16, kind="Internal",
                            addr_space="Shared")
    partial = nc.dram_tensor("partial", [SHARD, NGRP * OB], dt.float32,
                             kind="Internal")
    out = nc.dram_tensor("out", [SHARD, NGRP * OB], dt.float32,
                         kind="ExternalOutput")

    n_wchunks = 5 * D * BL // P  # 10

    with tile.TileContext(nc) as tc:
        with tc.tile_pool(name="const", bufs=1) as cpool:
            iota_sb = cpool.tile([P, P], dt.bfloat16)
            nc.sync.dma_start(iota_sb[:], iota_in[:, :])
            wt_sb = cpool.tile([P, n_wchunks, OB], dt.bfloat16)
            nc.sync.dma_start(
                wt_sb[:], wt_in[:, :].rearrange("(k p) o -> p k o", p=P))
            bias_sb = cpool.tile([P, OB], dt.float32)
            nc.sync.dma_start(bias_sb[:], bias_in[:, :])

            sup_sb = []
            for i, meta in enumerate((meta1, meta2)):
                ch = meta['ch']
                idx_sb = cpool.tile([P, ch * 8], dt.int16, name=f"idxsb{i}")
                nc.sync.dma_start(idx_sb[:], sup_t[i]['idx'][:, :])
                dst_sb = cpool.tile([P, ch], dt.bfloat16, name=f"dstsb{i}")
                nc.sync.dma_start(dst_sb[:], sup_t[i]['dst'][:, :])
                v_sb = cpool.tile([P, ch], dt.bfloat16, name=f"vsb{i}")
                nc.sync.dma_start(v_sb[:], sup_t[i]['v'][:, :])
                sup_sb.append((idx_sb, dst_sb, v_sb))

            xs = [x0loc, A1loc, R2loc, B1loc, R4loc]

            def emit_spmm(meta, sbufs, table_ap, ydst, zpool, selpool, evpool,
                          pspool, interleave=None):
                idx_sb, dst_sb, v_sb = sbufs
                ch = meta['ch']
                cb = meta['chunk_block']
                first = meta['first']
                last = meta['last']
                ps = None
                for call in range(ch // CALL):
                    z = zpool.tile([P, CALL, FEAT], dt.bfloat16, tag="z")
                    nc.gpsimd.dma_gather(
                        z[:, :, :], table_ap,
                        idx_sb[:, call * (CALL * 8):(call + 1) * (CALL * 8)],
                        CALL * P, CALL * P, FEAT, single_packet=False)
                    sel = selpool.tile([P, CALL, P], dt.bfloat16, tag="sel")
                    for g0 in range(0, CALL, SELG):
                        c0 = call * CALL + g0
                        sl = sel[:, g0:g0 + SELG, :]
                        nc.vector.tensor_tensor(
                            out=sl,
                            in0=iota_sb[:][:, None, :].to_broadcast(
                                [P, SELG, P]),
                            in1=dst_sb[:, c0:c0 + SELG, None].to_broadcast(
                                [P, SELG, P]),
                            op=mybir.AluOpType.is_equal)
                        nc.vector.tensor_tensor(
                            out=sl,
                            in0=sl,
                            in1=v_sb[:, c0:c0 + SELG, None].to_broadcast(
                                [P, SELG, P]),
                            op=mybir.AluOpType.mult)
                    for cl in range(CALL):
                        c = call * CALL + cl
                        if first[c]:
                            ps = [pspool.tile([P, SEC], dt.float32,
                                              tag=f"s{k}", name=f"ps{k}")
                                  for k in range(NSEC)]
                        for k in range(NSEC):
                            nc.tensor.matmul(
                                out=ps[k][:],
                                lhsT=sel[:, cl, :],
                                rhs=z[:, cl, k * SEC:(k + 1) * SEC],
                                start=bool(first[c]),
                                stop=bool(last[c]),
                            )
                        if last[c]:
                            b = cb[c]
                            y = evpool.tile([P, FEAT], dt.bfloat16, tag="y")
                            for k in range(NSEC):
                                nc.vector.tensor_copy(
                                    out=y[:, k * SEC:(k + 1) * SEC],
                                    in_=ps[k][:])
                            nc.sync.dma_start(
                                ydst[b * P:(b + 1) * P, :], y[:])
                    if interleave is not None:
                        interleave(call)

            def emit_proj_unit(t8, g, ks, pass2, lpool, psopool, opool,
                               ppool):
                rows = slice(t8 * TT * P, (t8 + 1) * TT * P)
                pso_banks = [psopool.tile([P, 2 * OB], dt.float32,
                                          tag=f"psop{i}", name=f"psop{i}")
                             for i in range(TT)]
                pso = [pso_banks[tt][:, :OB] for tt in range(TT)]
                for j, k in enumerate(ks):
                    m, h = divmod(k, 2)
                    lt = lpool.tile([P, TT * P], dt.bfloat16, tag="lt")
                    nc.sync.dma_start_transpose(
                        lt[:],
                        xs[m][rows, g * OB + h * P:g * OB + (h + 1) * P])
                    for tt in range(TT):
                        nc.tensor.matmul(
                            out=pso[tt],
                            lhsT=lt[:, tt * P:(tt + 1) * P],
                            rhs=wt_sb[:, k, :],
                            start=(j == 0),
                            stop=(j == len(ks) - 1),
                        )
                for tt in range(TT):
                    r2 = slice((t8 * TT + tt) * P, (t8 * TT + tt + 1) * P)
                    osb = opool.tile([P, OB], dt.float32, tag="osb")
                    if pass2:
                        par = ppool.tile([P, OB], dt.float32, tag="par")
                        nc.sync.dma_start(par[:],
                                          partial[r2, g * OB:(g + 1) * OB])
                        nc.vector.tensor_tensor(
                            out=osb[:], in0=pso[tt], in1=par[:],
                            op=mybir.AluOpType.add)
                        nc.sync.dma_start(out[r2, g * OB:(g + 1) * OB],
                                          osb[:])
                    else:
                        nc.vector.tensor_tensor(
                            out=osb[:], in0=pso[tt], in1=bias_sb[:],
                            op=mybir.AluOpType.add)
                        nc.sync.dma_start(partial[r2, g * OB:(g + 1) * OB],
                                          osb[:])

            rg = [list(range(NCORES))]
            with (
                tc.tile_pool(name="z", bufs=4) as zpool,
                tc.tile_pool(name="sel", bufs=4) as selpool,
                tc.tile_pool(name="ev", bufs=2) as evpool,
            ):
                with tc.tile_pool(name="ps", bufs=2, space="PSUM") as pspool:
                    pools = (zpool, selpool, evpool, pspool)
                    emit_spmm(meta1, sup_sb[0], x0full[:, :], A1loc, *pools)
                    nc.gpsimd.collective_compute(
                        "AllGather", mybir.AluOpType.bypass,
                        replica_groups=rg,
                        ins=[A1loc[:, :]], outs=[A1full[:, :]])
                    emit_spmm(meta2, sup_sb[1], A1full[:, :], B1loc, *pools)
                    nc.gpsimd.collective_compute(
                        "AllGather", mybir.AluOpType.bypass,
                        replica_groups=rg,
                        ins=[B1loc[:, :]], outs=[B1full[:, :]])
                    emit_spmm(meta1, sup_sb[0], A1full[:, :], R2loc, *pools)

                with (
                    tc.tile_pool(name="ps2", bufs=1, space="PSUM") as pspool2,
                    tc.tile_pool(name="pso", bufs=1, space="PSUM") as psopool,
                    tc.tile_pool(name="lhs", bufs=3) as lpool,
                    tc.tile_pool(name="po", bufs=2) as opool,
                    tc.tile_pool(name="par", bufs=4) as ppool,
                ):
                    units = [(t8, g) for t8 in range(SHARD // (TT * P))
                             for g in range(NGRP)]
                    uidx = [0]

                    def interleave(call):
                        if call % 2 == 1 and uidx[0] < len(units):
                            t8, g = units[uidx[0]]
                            uidx[0] += 1
                            emit_proj_unit(t8, g, range(6), False,
                                           lpool, psopool, opool, ppool)

                    emit_spmm(meta2, sup_sb[1], B1full[:, :], R4loc,
                              zpool, selpool, evpool, pspool2, interleave)
                    while uidx[0] < len(units):
                        t8, g = units[uidx[0]]
                        uidx[0] += 1
                        emit_proj_unit(t8, g, range(6), False,
                                       lpool, psopool, opool, ppool)
                    for t8, g in units:
                        emit_proj_unit(t8, g, range(6, n_wchunks), True,
                                       lpool, psopool, opool, ppool)

    nc.compile()
    return nc


# ---------------------------------------------------------------- entry

def prepare(inputs):
    """Host prep: returns (nc, in_maps)."""
    inputs_f32 = np.asarray(inputs['inputs'], dtype=np.float32)
    adj_vals = np.asarray(inputs['adj_vals'], dtype=np.float32)
    rows = np.asarray(inputs['rows']).astype(np.int64)
    cols = np.asarray(inputs['cols']).astype(np.int64)
    weights = np.asarray(inputs['weights'], dtype=np.float32)
    biases = np.asarray(inputs['biases'], dtype=np.float32)

    (m1, p1), (m2, p2) = preprocess(adj_vals, rows, cols)
    wt = build_wtilde(weights)
    x0 = make_x0_full(inputs_f32)
    iota = np.tile(np.arange(P, dtype=np.float32)[None, :], (P, 1)).astype(BF16)
    bias_rep = np.zeros((P, OB), np.float32)
    for o in range(OUT):
        bias_rep[:, o * BL:(o + 1) * BL] = biases[0, o]

    nc = build_program(m1, m2)

    in_maps = []
    for c in range(NCORES):
        in_maps.append(dict(
            x0full=x0, x0loc=np.ascontiguousarray(
                x0[c * SHARD:(c + 1) * SHARD]),
            iota=iota, wt=wt, bias=bias_rep,
            idx0=p1[c]['idx_w'], dst0=p1[c]['dst_t'], v0=p1[c]['v_t'],
            idx1=p2[c]['idx_w'], dst1=p2[c]['dst_t'], v1=p2[c]['v_t'],
        ))
    return nc, in_maps


def postprocess(results):
    """Per-core out [SHARD, NGRP*OB] f32 (col = g*256 + o*4 + b') ->
    full [B, N, OUT] f32."""
    out = np.zeros((BTOT, N, OUT), np.float32)
    for c in range(NCORES):
        oc = results[c]['out']
        v = oc.reshape(SHARD, NGRP, OUT, BL)      # [n, g, o, b']
        v = v.transpose(1, 3, 0, 2)               # [g, b', n, o]
        out[:, c * SHARD:(c + 1) * SHARD] = v.reshape(BTOT, SHARD, OUT)
    return out


def kernel(**inputs):
    nc, in_maps = prepare(inputs)
    res = run_bass_kernel_spmd(nc, in_maps, core_ids=list(range(NCORES)))
    return postprocess(res.results)
